# revision 4
# baseline (speedup 1.0000x reference)
"""CompressedActivation (compress -> decompress round trip) on 8 NeuronCores.

The reference's stable-argsort gather/scatter round trip is the identity on
x (every value, zero or not, is scattered back to its original position), so
the kernel is a row-sharded memory copy: each core DMA-copies its
(1024, 8192) f32 shard DRAM -> DRAM at HBM line rate. No communication.
"""

import numpy as np

import concourse.bass as bass
import concourse.mybir as mybir
from concourse.bass_utils import run_bass_kernel_spmd

N_CORES = 8
ROWS, COLS = 8192, 8192
SHARD_ROWS = ROWS // N_CORES  # 1024 rows, 32 MiB per core

_nc_cache = None


def build_nc():
    nc = bass.Bass()
    x = nc.declare_dram_parameter(
        "x", [SHARD_ROWS, COLS], mybir.dt.float32, isOutput=False
    )
    y = nc.declare_dram_parameter(
        "out", [SHARD_ROWS, COLS], mybir.dt.float32, isOutput=True
    )
    # Pair up rows so each DMA descriptor is the 64KB max (16384 f32), the
    # most bandwidth-efficient shape measured for this DRAM->DRAM copy.
    # Split across the three DMA issue paths (two HWDGE rings + SWDGE) —
    # interleaving queue rows lands the fast HBM mode more reliably under
    # 8-core contention than a single instruction.
    x2 = x.rearrange("(p q) b -> p (q b)", q=2)
    y2 = y.rearrange("(p q) b -> p (q b)", q=2)
    a, b = 224, 448  # sync 7/16, scalar 7/16, gpsimd 2/16 of 512 rows
    with (
        nc.Block() as block,
        nc.semaphore("dma_sem") as dma_sem,
    ):
        @block.sync
        def _(sync):
            sync.dma_start(out=y2[:a], in_=x2[:a]).then_inc(dma_sem, 16)
            sync.wait_ge(dma_sem, 48)

        @block.scalar
        def _(scalar):
            scalar.dma_start(out=y2[a:b], in_=x2[a:b]).then_inc(dma_sem, 16)
            scalar.wait_ge(dma_sem, 48)

        @block.gpsimd
        def _(gpsimd):
            gpsimd.dma_start(out=y2[b:], in_=x2[b:]).then_inc(dma_sem, 16)
            gpsimd.wait_ge(dma_sem, 48)
    return nc


def kernel(x: np.ndarray) -> np.ndarray:
    global _nc_cache
    x = np.ascontiguousarray(x, dtype=np.float32)
    assert x.shape == (ROWS, COLS)
    if _nc_cache is None:
        _nc_cache = build_nc()
    in_maps = [
        {"x": x[i * SHARD_ROWS : (i + 1) * SHARD_ROWS]} for i in range(N_CORES)
    ]
    res = run_bass_kernel_spmd(_nc_cache, in_maps, core_ids=list(range(N_CORES)))
    out = np.empty((ROWS, COLS), dtype=np.float32)
    for i, r in enumerate(res.results):
        out[i * SHARD_ROWS : (i + 1) * SHARD_ROWS] = r["out"]
    return out


# revision 5
# speedup vs baseline: 1.1441x; 1.1441x over previous
"""CompressedActivation (compress -> decompress round trip) on 8 NeuronCores.

The reference's stable-argsort gather/scatter round trip is the identity on
x (every value, zero or not, is scattered back to its original position), so
the kernel is a row-sharded memory copy: each core DMA-copies its
(1024, 8192) f32 shard DRAM -> DRAM at HBM line rate. No communication.
"""

import numpy as np

import concourse.bass as bass
import concourse.mybir as mybir
from concourse.bass_utils import run_bass_kernel_spmd

N_CORES = 8
ROWS, COLS = 8192, 8192
SHARD_ROWS = ROWS // N_CORES  # 1024 rows, 32 MiB per core

_nc_cache = None


def build_nc():
    nc = bass.Bass()
    x = nc.declare_dram_parameter(
        "x", [SHARD_ROWS, COLS], mybir.dt.float32, isOutput=False
    )
    y = nc.declare_dram_parameter(
        "out", [SHARD_ROWS, COLS], mybir.dt.float32, isOutput=True
    )
    # Pair up rows so each DMA descriptor is the 64KB max (16384 f32), the
    # most bandwidth-efficient shape measured for this DRAM->DRAM copy.
    # 16 interleaved chunks alternating between the two HWDGE rings
    # (sync/scalar) — address-interleaved queue traffic was the most robust
    # structure against cross-core HBM contention in profiling.
    x2 = x.rearrange("(p q) b -> p (q b)", q=2)
    y2 = y.rearrange("(p q) b -> p (q b)", q=2)
    R, C = 512, 32  # 16 chunks of 32 rows (2 MiB each)
    with (
        nc.Block() as block,
        nc.semaphore("dma_sem") as dma_sem,
    ):
        @block.sync
        def _(sync):
            for i in range(0, 16, 2):
                sync.dma_start(
                    out=y2[i * C : (i + 1) * C], in_=x2[i * C : (i + 1) * C]
                ).then_inc(dma_sem, 16)
            sync.wait_ge(dma_sem, 256)

        @block.scalar
        def _(scalar):
            for i in range(1, 16, 2):
                scalar.dma_start(
                    out=y2[i * C : (i + 1) * C], in_=x2[i * C : (i + 1) * C]
                ).then_inc(dma_sem, 16)
            scalar.wait_ge(dma_sem, 256)
    return nc


def kernel(x: np.ndarray) -> np.ndarray:
    global _nc_cache
    x = np.ascontiguousarray(x, dtype=np.float32)
    assert x.shape == (ROWS, COLS)
    if _nc_cache is None:
        _nc_cache = build_nc()
    in_maps = [
        {"x": x[i * SHARD_ROWS : (i + 1) * SHARD_ROWS]} for i in range(N_CORES)
    ]
    res = run_bass_kernel_spmd(_nc_cache, in_maps, core_ids=list(range(N_CORES)))
    out = np.empty((ROWS, COLS), dtype=np.float32)
    for i, r in enumerate(res.results):
        out[i * SHARD_ROWS : (i + 1) * SHARD_ROWS] = r["out"]
    return out
